# revision 49
# baseline (speedup 1.0000x reference)
"""CAM (channel attention module) kernel for Trainium2, 8-core SPMD.

Problem: x (16, 512, 64, 64) f32, gamma (1,) f32.
  v = x.reshape(B, C, N);  E = v @ v.T  (B x 512 x 512)
  att = softmax(rowmax(E) - E)  ==  exp(rowmin(E) - E) / rowsum(...)
  out = gamma * (att @ v) + x

Sharding: data-parallel over batch, 2 batches per core, no collectives.

Per-core per-batch pipeline (v6):
  T: v16 (fp16) quarter tiles load straight from HBM with gpsimd
     casting DMAs (the SWDGE converts dtype in flight), q-major so
     each energy chunk becomes complete as early as possible; 128 PE
     transposes (fp16) + DVE/ACT copies build the per-quarter vT
     tiles [128, 8, 4, 128].  v8 (fp8e4m3) loads one whole channel
     row per casting DMA.
  E: energy = vT.T @ vT, fp16 operands, f32 PSUM accumulation.
     Symmetry: row-tile ct computes only columns >= ct*128; the 6
     missing blocks are mirrored from earlier rows by PE transpose
     into the PSUM row.
  S: rowmin (DVE), then one ACT exp with accumulated f32 rowsum
     writing the unscaled fp16 attention weights (max entry exactly
     1.0 per row); gamma/rowsum stays a per-row f32 scalar rg applied
     in the epilogue, so gamma==0 zeroes the attention term exactly.
  A: per-row-block attT: right after row ct's exp, 4 PE transposes
     build atp[ct] and one ACT casting copy emits the per-ct fp8
     attT tile [128, dt, 128] -- O tiles for out-rows ct depend only
     on row ct's softmax, so the O phase streams per-ct instead of
     waiting for the whole S phase.
  O: out = attT.T @ v8 with fp8 DoubleRow matmuls (0.5 PE cycles per
     output row, contracting 256 channels per instruction) into f32
     PSUM, ct-major so each row block drains as its attention lands;
     the epilogue fuses (psum * rg) + x16 in one DVE
     scalar_tensor_tensor per tile (gpsimd cannot read PSUM on
     hardware); stores spread across the sync/scalar/gpsimd queues.
  Next-batch v16 loads are emitted before this batch's v8 loads so
     the b+1 energy fill is not queued behind them on gpsimd.
  The fp8 residual-correction pass of v1 is dropped: attention-path
     precision is fp8-grade (~1.6e-2 rel at gamma=0.37, under the
     2e-2 bar); the x residual rides on the fp16 copy of x (~4e-4
     rel at gamma=0, the regime setup_inputs() pins).
"""
import sys

import numpy as np

if "/opt/trn_rl_repo" not in sys.path:
    sys.path.insert(0, "/opt/trn_rl_repo")

import concourse.bass as bass
import concourse.tile as tile
from concourse import bacc, mybir
from concourse.bass_utils import run_bass_kernel_spmd
from concourse.masks import make_identity

N_CORES = 8
B_FULL = 16
B_PER_CORE = B_FULL // N_CORES  # 2
C = 512            # channels
HW = 4096          # H*W
CT = C // 128      # 4 channel tiles
QW = HW // 4       # quarter of H*W
NCH = HW // 512    # 8 output column chunks (512 wide)

f32 = mybir.dt.float32
f16 = mybir.dt.float16
f8 = mybir.dt.float8e4

_CACHE = {}

DR = mybir.MatmulPerfMode.DoubleRow


def _build_nc():
    nc = bacc.Bacc(None, target_bir_lowering=False)
    x_d = nc.dram_tensor("x", [B_PER_CORE, C, HW], f32, kind="ExternalInput")
    g_d = nc.dram_tensor("gamma", [1], f32, kind="ExternalInput")
    y_d = nc.dram_tensor("y", [B_PER_CORE, C, HW], f32, kind="ExternalOutput")

    with tile.TileContext(nc) as tc:
        with (
            tc.tile_pool(name="pxq", bufs=2) as pxq,        # f32 x staging
            tc.tile_pool(name="pv16", bufs=2) as pv16,      # fp16 v quarters
            tc.tile_pool(name="pvt", bufs=2) as pvt,        # vT quarter tiles
            tc.tile_pool(name="pv8", bufs=2) as pv8,        # fp8 v pairs
            tc.tile_pool(name="patt", bufs=1) as patt,      # att8 / attT8
            tc.tile_pool(name="pstage", bufs=4) as pstage,  # out staging
            tc.tile_pool(name="psmall", bufs=8) as psmall,  # per-ct scalars
            tc.tile_pool(name="pmir", bufs=1) as pmir,      # mirror blocks
            tc.tile_pool(name="psing", bufs=1) as psing,    # ident, gamma
            tc.tile_pool(name="ptp", bufs=2, space="PSUM") as ptp,
            tc.tile_pool(name="pep", bufs=2, space="PSUM") as pep,
            tc.tile_pool(name="pop", bufs=3, space="PSUM") as pop,
            tc.tile_pool(name="pat", bufs=1, space="PSUM") as pat,
        ):
            ident = psing.tile([128, 128], f32)
            make_identity(nc, ident)
            ident16 = psing.tile([128, 128], f16)
            nc.vector.tensor_copy(out=ident16, in_=ident)
            gam = psing.tile([128, 1], f32)

            def load_gamma():
                g_ap = g_d[:]
                nc.gpsimd.dma_start(
                    out=gam,
                    in_=bass.AP(tensor=g_ap.tensor, offset=g_ap.offset,
                                ap=[[0, 128], [1, 1]]),
                )

            def load_v16(b):
                """fp16 casting loads of x straight from HBM, then PE
                transposes into vT quarter tiles (PSUM roundtrip)."""
                tiles = [[None] * 4 for _ in range(CT)]
                v8p = v8_tiles()
                vTq = [pvt.tile([128, 8, CT, 128], f16, tag=f"vTq{q}",
                                name=f"vTq{q}") for q in range(4)]
                for i, (ct, q) in enumerate(
                        (ct, q) for q in range(4) for ct in range(CT)):
                    t_ = pv16.tile([128, QW], f16, tag=f"v16_{ct}q{q}",
                                   name=f"v16_{ct}q{q}")
                    nc.gpsimd.dma_start(
                        out=t_,
                        in_=x_d[b, ct * 128:(ct + 1) * 128,
                                q * QW:(q + 1) * QW],
                    )
                    tp = ptp.tile([128, 8, 128], f16, tag="tp")
                    for ks in range(8):
                        nc.tensor.transpose(
                            tp[:, ks, :],
                            t_[:, ks * 128:(ks + 1) * 128],
                            ident16,
                        )
                    if i % 2:
                        nc.vector.tensor_copy(out=vTq[q][:, :, ct, :],
                                              in_=tp)
                    else:
                        nc.scalar.copy(out=vTq[q][:, :, ct, :], in_=tp)
                    tiles[ct][q] = t_
                return tiles, vTq, v8p

            def emit_v8(b, v8p):
                """fp8 casting loads, one channel row each.  Emitted AFTER
                the next batch's v16 loads so the fill of batch b+1's
                energy pipeline is not stuck behind them on gpsimd."""
                for g in range(2):
                    for j in range(2):
                        ct = 2 * g + j
                        nc.gpsimd.dma_start(
                            out=v8p[g][:, j, :],
                            in_=x_d[b, ct * 128:(ct + 1) * 128, :],
                        )

            def v8_tiles():
                return [pv8.tile([128, 2, HW], f8, tag=f"v8_{g}",
                                 name=f"v8_{g}") for g in range(2)]

            def load_v8_slice(b, v8p, g, j, q):
                ct = 2 * g + j
                nc.gpsimd.dma_start(
                    out=v8p[g][:, j, q * QW:(q + 1) * QW],
                    in_=x_d[b, ct * 128:(ct + 1) * 128, q * QW:(q + 1) * QW],
                )

            batches = list(range(B_PER_CORE))
            v16, vTq, v8p = load_v16(batches[0])
            load_gamma()

            v16_next = vTq_next = v8p_next = None

            for bi, b in enumerate(batches):
                if bi > 0:
                    v16, vTq, v8p = v16_next, vTq_next, v8p_next

                def vT(k):
                    return vTq[k // 8][:, k % 8, :, :]

                # ---- E + S: energy (fp16, f32 accum) + softmax ----
                att8 = [None] * CT
                rg2 = [None] * CT
                attT = [None] * CT
                mirror_src = {}
                for ct in range(CT):
                    off = ct * 128
                    ep = pep.tile([128, C], f32, tag="ep")
                    for k in range(32):
                        nc.tensor.matmul(
                            ep[:, off:],
                            lhsT=vT(k)[:, ct, :],
                            rhs=vT(k)[:, ct:, :],
                            start=(k == 0),
                            stop=(k == 31),
                        )
                    for (dst, src) in (((1, 0), (0, 1)), ((2, 0), (0, 2)),
                                       ((2, 1), (1, 2)), ((3, 0), (0, 3)),
                                       ((3, 1), (1, 3)), ((3, 2), (2, 3))):
                        if src[0] == ct:
                            sb = pmir.tile([128, 128], f32,
                                           tag=f"mir{dst[0]}{dst[1]}")
                            nc.scalar.copy(
                                out=sb,
                                in_=ep[:, src[1] * 128:(src[1] + 1) * 128],
                            )
                            mirror_src[dst] = sb
                    for dt in range(ct):
                        nc.tensor.transpose(
                            ep[:, dt * 128:(dt + 1) * 128],
                            mirror_src[(ct, dt)], ident,
                        )
                    mn = psmall.tile([128, 1], f32, tag="mn")
                    nc.vector.tensor_reduce(
                        out=mn, in_=ep, axis=mybir.AxisListType.X,
                        op=mybir.AluOpType.min,
                    )
                    a_ = patt.tile([128, C], f16, tag=f"att{ct}")
                    ss = psmall.tile([128, 1], f32, tag="ss")
                    nc.scalar.activation(
                        out=a_, in_=ep,
                        func=mybir.ActivationFunctionType.Exp,
                        bias=mn, scale=-1.0, accum_out=ss,
                    )
                    rg = psmall.tile([128, 1], f32, tag=f"rg{ct}")
                    nc.vector.reciprocal(out=rg, in_=ss)
                    nc.vector.tensor_mul(out=rg, in0=rg, in1=gam)
                    att8[ct] = a_
                    rg2[ct] = rg
                    # A(ct): transpose and cast this row's attention blocks
                    # immediately -- O tiles for out-rows ct depend only on
                    # this row's softmax, so the O phase streams per-ct
                    # instead of waiting for the whole S phase.
                    atp = pat.tile([128, CT, 128], f16, tag="atp",
                                   name=f"atp{ct}")
                    for dt in range(CT):
                        nc.tensor.transpose(
                            atp[:, dt, :],
                            a_[:, dt * 128:(dt + 1) * 128],
                            ident16,
                        )
                    attT[ct] = patt.tile([128, CT, 128], f8,
                                         tag=f"attTc{ct}", name=f"attTc{ct}")
                    nc.scalar.copy(out=attT[ct], in_=atp)

                # prefetch next batch's T chain after the A phase; this
                # batch's v8 loads queue behind the next batch's v16 loads
                if bi + 1 < len(batches):
                    v16_next, vTq_next, v8p_next = load_v16(batches[bi + 1])
                    emit_v8(b, v8p)
                    emit_v8(batches[bi + 1], v8p_next)
                elif bi == 0:
                    emit_v8(b, v8p)
                else:
                    v16_next = vTq_next = v8p_next = None

                # ---- O: DoubleRow fp8 out = attT.T @ v8 ----
                last = v16_next is None
                for ct in range(CT):
                    for n in range(NCH):
                        nsl = slice(n * 512, (n + 1) * 512)
                        op = pop.tile([128, 2, 256], f32, tag="op")
                        for h in range(2):
                            hsl = slice(n * 512 + h * 256,
                                        n * 512 + (h + 1) * 256)
                            for g in range(2):
                                nc.tensor.matmul(
                                    op[:, h, :],
                                    lhsT=attT[ct][:, 2 * g:2 * g + 2, :],
                                    rhs=v8p[g][:, :, hsl],
                                    start=(g == 0),
                                    stop=(g == 1),
                                    perf_mode=DR,
                                )
                        st = pstage.tile([128, 512], f32, tag="st")
                        nc.vector.scalar_tensor_tensor(
                            out=st,
                            in0=op,
                            scalar=rg2[ct],
                            in1=v16[ct][2 * n // 4][
                                :, (n * 512) % QW:(n * 512) % QW + 512],
                            op0=mybir.AluOpType.mult,
                            op1=mybir.AluOpType.add,
                        )
                        i = ct * NCH + n
                        if last:
                            # gpsimd is idle in the last batch's O phase
                            seng = (nc.sync, nc.scalar, nc.gpsimd)[i % 3]
                        else:
                            seng = (nc.sync, nc.scalar, nc.sync)[i % 3]
                        seng.dma_start(
                            out=y_d[b, ct * 128:(ct + 1) * 128, nsl], in_=st,
                        )

    nc.compile()
    return nc


def kernel(x: np.ndarray, gamma: np.ndarray) -> np.ndarray:
    x = np.ascontiguousarray(np.asarray(x, dtype=np.float32))
    gamma = np.ascontiguousarray(np.asarray(gamma, dtype=np.float32))
    B, Cc, H, W = x.shape
    xv = x.reshape(B, Cc, H * W)

    if "nc" not in _CACHE:
        _CACHE["nc"] = _build_nc()
    nc = _CACHE["nc"]

    in_maps = [
        {"x": xv[i * B_PER_CORE:(i + 1) * B_PER_CORE], "gamma": gamma}
        for i in range(N_CORES)
    ]
    res = run_bass_kernel_spmd(nc, in_maps, list(range(N_CORES)))
    y = np.concatenate([res.results[i]["y"] for i in range(N_CORES)], axis=0)
    return y.reshape(B, Cc, H, W).astype(np.float32)


# revision 55
# speedup vs baseline: 1.0166x; 1.0166x over previous
"""CAM (channel attention module) kernel for Trainium2, 8-core SPMD.

Problem: x (16, 512, 64, 64) f32, gamma (1,) f32.
  v = x.reshape(B, C, N);  E = v @ v.T  (B x 512 x 512)
  att = softmax(rowmax(E) - E)  ==  exp(rowmin(E) - E) / rowsum(...)
  out = gamma * (att @ v) + x

Sharding: data-parallel over batch, 2 batches per core, no collectives.

Per-core per-batch pipeline (v6):
  T: v16 (fp16) quarter tiles load straight from HBM with gpsimd
     casting DMAs (the SWDGE converts dtype in flight), q-major so
     each energy chunk becomes complete as early as possible; 128 PE
     transposes (fp16) + DVE/ACT copies build the per-quarter vT
     tiles [128, 8, 4, 128].  v8 (fp8e4m3) loads one whole channel
     row per casting DMA.
  E: energy = vT.T @ vT, fp16 operands, f32 PSUM accumulation.
     Symmetry: row-tile ct computes only columns >= ct*128; the 6
     missing blocks are mirrored from earlier rows by PE transpose
     into the PSUM row.
  S: rowmin (DVE), then one ACT exp with accumulated f32 rowsum
     writing the unscaled fp16 attention weights (max entry exactly
     1.0 per row); gamma/rowsum stays a per-row f32 scalar rg applied
     in the epilogue, so gamma==0 zeroes the attention term exactly.
  A: per-row-block attT: right after row ct's exp, 4 PE transposes
     build atp[ct] and one ACT casting copy emits the per-ct fp8
     attT tile [128, dt, 128] -- O tiles for out-rows ct depend only
     on row ct's softmax, so the O phase streams per-ct instead of
     waiting for the whole S phase.
  O: out = attT.T @ v8 with fp8 DoubleRow matmuls (0.5 PE cycles per
     output row, contracting 256 channels per instruction) into f32
     PSUM, ct-major so each row block drains as its attention lands;
     the epilogue fuses (psum * rg) + x16 in one DVE
     scalar_tensor_tensor per tile (gpsimd cannot read PSUM on
     hardware); stores spread across the sync/scalar/gpsimd queues.
  Next-batch v16 loads are emitted before this batch's v8 loads so
     the b+1 energy fill is not queued behind them on gpsimd.
  The fp8 residual-correction pass of v1 is dropped: attention-path
     precision is fp8-grade (~1.6e-2 rel at gamma=0.37, under the
     2e-2 bar); the x residual rides on the fp16 copy of x (~4e-4
     rel at gamma=0, the regime setup_inputs() pins).
"""
import sys

import numpy as np

if "/opt/trn_rl_repo" not in sys.path:
    sys.path.insert(0, "/opt/trn_rl_repo")

import concourse.bass as bass
import concourse.tile as tile
from concourse import bacc, mybir
from concourse.bass_utils import run_bass_kernel_spmd
from concourse.masks import make_identity

N_CORES = 8
B_FULL = 16
B_PER_CORE = B_FULL // N_CORES  # 2
C = 512            # channels
HW = 4096          # H*W
CT = C // 128      # 4 channel tiles
QW = HW // 4       # quarter of H*W
NCH = HW // 512    # 8 output column chunks (512 wide)

f32 = mybir.dt.float32
f16 = mybir.dt.float16
f8 = mybir.dt.float8e4

_CACHE = {}

DR = mybir.MatmulPerfMode.DoubleRow


def _build_nc():
    nc = bacc.Bacc(None, target_bir_lowering=False)
    x_d = nc.dram_tensor("x", [B_PER_CORE, C, HW], f32, kind="ExternalInput")
    g_d = nc.dram_tensor("gamma", [1], f32, kind="ExternalInput")
    y_d = nc.dram_tensor("y", [B_PER_CORE, C, HW], f32, kind="ExternalOutput")

    with tile.TileContext(nc) as tc:
        with (
            tc.tile_pool(name="pxq", bufs=2) as pxq,        # f32 x staging
            tc.tile_pool(name="pv16", bufs=2) as pv16,      # fp16 v quarters
            tc.tile_pool(name="pvt", bufs=2) as pvt,        # vT quarter tiles
            tc.tile_pool(name="pv8", bufs=2) as pv8,        # fp8 v pairs
            tc.tile_pool(name="patt", bufs=1) as patt,      # att8 / attT8
            tc.tile_pool(name="pstage", bufs=5) as pstage,  # out staging
            tc.tile_pool(name="psmall", bufs=8) as psmall,  # per-ct scalars
            tc.tile_pool(name="pmir", bufs=1) as pmir,      # mirror blocks
            tc.tile_pool(name="psing", bufs=1) as psing,    # ident, gamma
            tc.tile_pool(name="ptp", bufs=2, space="PSUM") as ptp,
            tc.tile_pool(name="pep", bufs=2, space="PSUM") as pep,
            tc.tile_pool(name="pop", bufs=3, space="PSUM") as pop,
            tc.tile_pool(name="pat", bufs=1, space="PSUM") as pat,
        ):
            ident = psing.tile([128, 128], f32)
            make_identity(nc, ident)
            ident16 = psing.tile([128, 128], f16)
            nc.vector.tensor_copy(out=ident16, in_=ident)
            gam = psing.tile([128, 1], f32)

            def load_gamma():
                g_ap = g_d[:]
                nc.gpsimd.dma_start(
                    out=gam,
                    in_=bass.AP(tensor=g_ap.tensor, offset=g_ap.offset,
                                ap=[[0, 128], [1, 1]]),
                )

            def load_v16(b):
                """fp16 casting loads of x straight from HBM, then PE
                transposes into vT quarter tiles (PSUM roundtrip)."""
                tiles = [[None] * 4 for _ in range(CT)]
                v8p = v8_tiles()
                vTq = [pvt.tile([128, 8, CT, 128], f16, tag=f"vTq{q}",
                                name=f"vTq{q}") for q in range(4)]
                for i, (ct, q) in enumerate(
                        (ct, q) for q in range(4) for ct in range(CT)):
                    t_ = pv16.tile([128, QW], f16, tag=f"v16_{ct}q{q}",
                                   name=f"v16_{ct}q{q}")
                    nc.gpsimd.dma_start(
                        out=t_,
                        in_=x_d[b, ct * 128:(ct + 1) * 128,
                                q * QW:(q + 1) * QW],
                    )
                    tp = ptp.tile([128, 8, 128], f16, tag="tp")
                    for ks in range(8):
                        nc.tensor.transpose(
                            tp[:, ks, :],
                            t_[:, ks * 128:(ks + 1) * 128],
                            ident16,
                        )
                    if i % 2:
                        nc.vector.tensor_copy(out=vTq[q][:, :, ct, :],
                                              in_=tp)
                    else:
                        nc.scalar.copy(out=vTq[q][:, :, ct, :], in_=tp)
                    tiles[ct][q] = t_
                return tiles, vTq, v8p

            def emit_v8(b, v8p):
                """fp8 casting loads, one channel row each.  Emitted AFTER
                the next batch's v16 loads so the fill of batch b+1's
                energy pipeline is not stuck behind them on gpsimd."""
                for g in range(2):
                    for j in range(2):
                        ct = 2 * g + j
                        nc.gpsimd.dma_start(
                            out=v8p[g][:, j, :],
                            in_=x_d[b, ct * 128:(ct + 1) * 128, :],
                        )

            def v8_tiles():
                return [pv8.tile([128, 2, HW], f8, tag=f"v8_{g}",
                                 name=f"v8_{g}") for g in range(2)]

            def load_v8_slice(b, v8p, g, j, q):
                ct = 2 * g + j
                nc.gpsimd.dma_start(
                    out=v8p[g][:, j, q * QW:(q + 1) * QW],
                    in_=x_d[b, ct * 128:(ct + 1) * 128, q * QW:(q + 1) * QW],
                )

            batches = list(range(B_PER_CORE))
            v16, vTq, v8p = load_v16(batches[0])
            load_gamma()

            v16_next = vTq_next = v8p_next = None

            for bi, b in enumerate(batches):
                if bi > 0:
                    v16, vTq, v8p = v16_next, vTq_next, v8p_next

                def vT(k):
                    return vTq[k // 8][:, k % 8, :, :]

                # ---- E + S: energy (fp16, f32 accum) + softmax ----
                att8 = [None] * CT
                rg2 = [None] * CT
                attT = [None] * CT
                mirror_src = {}
                for ct in range(CT):
                    off = ct * 128
                    ep = pep.tile([128, C], f32, tag="ep")
                    for k in range(32):
                        nc.tensor.matmul(
                            ep[:, off:],
                            lhsT=vT(k)[:, ct, :],
                            rhs=vT(k)[:, ct:, :],
                            start=(k == 0),
                            stop=(k == 31),
                        )
                    for (dst, src) in (((1, 0), (0, 1)), ((2, 0), (0, 2)),
                                       ((2, 1), (1, 2)), ((3, 0), (0, 3)),
                                       ((3, 1), (1, 3)), ((3, 2), (2, 3))):
                        if src[0] == ct:
                            sb = pmir.tile([128, 128], f32,
                                           tag=f"mir{dst[0]}{dst[1]}")
                            nc.scalar.copy(
                                out=sb,
                                in_=ep[:, src[1] * 128:(src[1] + 1) * 128],
                            )
                            mirror_src[dst] = sb
                    for dt in range(ct):
                        nc.tensor.transpose(
                            ep[:, dt * 128:(dt + 1) * 128],
                            mirror_src[(ct, dt)], ident,
                        )
                    mn = psmall.tile([128, 1], f32, tag="mn")
                    nc.vector.tensor_reduce(
                        out=mn, in_=ep, axis=mybir.AxisListType.X,
                        op=mybir.AluOpType.min,
                    )
                    a_ = patt.tile([128, C], f16, tag=f"att{ct}")
                    ss = psmall.tile([128, 1], f32, tag="ss")
                    nc.scalar.activation(
                        out=a_, in_=ep,
                        func=mybir.ActivationFunctionType.Exp,
                        bias=mn, scale=-1.0, accum_out=ss,
                    )
                    rg = psmall.tile([128, 1], f32, tag=f"rg{ct}")
                    nc.vector.reciprocal(out=rg, in_=ss)
                    nc.vector.tensor_mul(out=rg, in0=rg, in1=gam)
                    att8[ct] = a_
                    rg2[ct] = rg
                    # A(ct): transpose and cast this row's attention blocks
                    # immediately -- O tiles for out-rows ct depend only on
                    # this row's softmax, so the O phase streams per-ct
                    # instead of waiting for the whole S phase.
                    atp = pat.tile([128, CT, 128], f16, tag="atp",
                                   name=f"atp{ct}")
                    for dt in range(CT):
                        nc.tensor.transpose(
                            atp[:, dt, :],
                            a_[:, dt * 128:(dt + 1) * 128],
                            ident16,
                        )
                    attT[ct] = patt.tile([128, CT, 128], f8,
                                         tag=f"attTc{ct}", name=f"attTc{ct}")
                    nc.scalar.copy(out=attT[ct], in_=atp)

                # prefetch next batch's T chain after the A phase; this
                # batch's v8 loads queue behind the next batch's v16 loads
                if bi + 1 < len(batches):
                    v16_next, vTq_next, v8p_next = load_v16(batches[bi + 1])
                    emit_v8(b, v8p)
                    emit_v8(batches[bi + 1], v8p_next)
                elif bi == 0:
                    emit_v8(b, v8p)
                else:
                    v16_next = vTq_next = v8p_next = None

                # ---- O: DoubleRow fp8 out = attT.T @ v8 ----
                last = v16_next is None
                for ct in range(CT):
                    for n in range(NCH):
                        nsl = slice(n * 512, (n + 1) * 512)
                        op = pop.tile([128, 2, 256], f32, tag="op")
                        for h in range(2):
                            hsl = slice(n * 512 + h * 256,
                                        n * 512 + (h + 1) * 256)
                            for g in range(2):
                                nc.tensor.matmul(
                                    op[:, h, :],
                                    lhsT=attT[ct][:, 2 * g:2 * g + 2, :],
                                    rhs=v8p[g][:, :, hsl],
                                    start=(g == 0),
                                    stop=(g == 1),
                                    perf_mode=DR,
                                )
                        st = pstage.tile([128, 512], f32, tag="st")
                        nc.vector.scalar_tensor_tensor(
                            out=st,
                            in0=op,
                            scalar=rg2[ct],
                            in1=v16[ct][2 * n // 4][
                                :, (n * 512) % QW:(n * 512) % QW + 512],
                            op0=mybir.AluOpType.mult,
                            op1=mybir.AluOpType.add,
                        )
                        i = ct * NCH + n
                        if last:
                            # gpsimd is idle in the last batch's O phase
                            seng = (nc.gpsimd, nc.sync, nc.scalar)[i % 3]
                        else:
                            seng = (nc.sync, nc.scalar, nc.sync)[i % 3]
                        seng.dma_start(
                            out=y_d[b, ct * 128:(ct + 1) * 128, nsl], in_=st,
                        )

    nc.compile()
    return nc


def kernel(x: np.ndarray, gamma: np.ndarray) -> np.ndarray:
    x = np.ascontiguousarray(np.asarray(x, dtype=np.float32))
    gamma = np.ascontiguousarray(np.asarray(gamma, dtype=np.float32))
    B, Cc, H, W = x.shape
    xv = x.reshape(B, Cc, H * W)

    if "nc" not in _CACHE:
        _CACHE["nc"] = _build_nc()
    nc = _CACHE["nc"]

    in_maps = [
        {"x": xv[i * B_PER_CORE:(i + 1) * B_PER_CORE], "gamma": gamma}
        for i in range(N_CORES)
    ]
    res = run_bass_kernel_spmd(nc, in_maps, list(range(N_CORES)))
    y = np.concatenate([res.results[i]["y"] for i in range(N_CORES)], axis=0)
    return y.reshape(B, Cc, H, W).astype(np.float32)
